# revision 1
# baseline (speedup 1.0000x reference)
"""Trainium2 Bass kernel for nn_CPSN (retrieval_knn PSM/PWG module).

Contract: kernel(**inputs) takes the FULL unsharded inputs (as produced by
setup_inputs) and returns the FULL output [2, b*q, s], distributing work
across 8 NeuronCores internally (data-parallel over the query dim q).

Algorithm per (q, s) pair (b=1, s=25, q=30, c=512, hw=361):
  O[x, y] = <f2n[:, x], f1n[:, y]>   (x = query pixel, y = support pixel)
  s21[x] = max_y O ; s12[y] = max_x O
  g1[x] = a1[argmax_y O[x, :]] ; g2[y] = a2[argmax_x O[:, y]]
  w = g1 * g2 ; out0 = mean(s12 * w) ; out1 = mean(s21 * w)

On device, argmax+gather is computed WITHOUT indices: the one-hot row
(O[x, :] == s21[x]) dotted with the (partition-broadcast) attention row a1
is exactly g1[x].  That is a single fused DVE pass:
  scalar_tensor_tensor(out, in0=O, scalar=s21col, in1=a1bc,
                       op0=is_ge, op1=mult, accum_out=g1col)
Both orientations are computed by swapping matmul operands (PE contracts the
partition dim only).  The meta-learner (two 1x1 convs + BN + ReLU, ~1% of
FLOPs) runs on host; its output rows are fed pre-broadcast to the device.
"""

import os
import sys

import numpy as np

for _p in ("/opt/trn_rl_repo", "/root/.axon_site/_ro/trn_rl_repo"):
    if os.path.isdir(_p) and _p not in sys.path:
        sys.path.insert(0, _p)

import concourse.bass as bass
import concourse.tile as tile
from concourse import bacc, library_config, mybir
from concourse.bass_utils import run_bass_kernel_spmd

# ---- problem constants (hardcoded per contract) ----
B, S, Q, C, H, W, TEMP = 1, 25, 30, 512, 19, 19, 64
HW = H * W  # 361
NCORES = 8
L = 4               # local (padded) query images per core; Q_PAD = 32
Q_PAD = NCORES * L
CCH = C // 128      # 4 contraction chunks
PCH = [(0, 128), (128, 128), (256, HW - 256)]  # pixel-dim partition chunks
BLOCKS = [(0, 13), (13, 12)]  # ss blocking to bound SBUF residency
GRP = 4             # O-phase ss group size (PSUM bank budget)
BN_EPS = 1e-5

F32 = mybir.dt.float32
AX_X = mybir.AxisListType.X
OP = mybir.AluOpType
AF = mybir.ActivationFunctionType


def _col_off(l, kind, pch, ss):
    # cols2d free layout: [L][kind:4][pchunk:3][S]
    return ((l * 4 + kind) * 3 + pch) * S + ss


def build_program(variant="", repeat=1):
    """Build the (SPMD-shared) single-core bass program."""
    nc = bacc.Bacc(None, target_bir_lowering=False, debug=False)

    f1_d = nc.dram_tensor("f1", [S, C, HW], F32, kind="ExternalInput")
    f2_d = nc.dram_tensor("f2s", [L, C, HW], F32, kind="ExternalInput")
    # attention rows, host-gathered per (l, ss); broadcast to 128 rows on-device
    a1r_d = nc.dram_tensor("a1r", [L, S, HW], F32, kind="ExternalInput")
    a2r_d = nc.dram_tensor("a2r", [L, HW], F32, kind="ExternalInput")
    out_d = nc.dram_tensor("out", [2 * L, S], F32, kind="ExternalOutput")

    with tile.TileContext(nc) as tc:
        from contextlib import ExitStack

        with ExitStack() as ctx:
            pp = ctx.enter_context(tc.tile_pool(name="pp", bufs=2, space="PSUM"))
            f1n_pool = ctx.enter_context(tc.tile_pool(name="f1n", bufs=13 * CCH))
            f2n_pool = ctx.enter_context(tc.tile_pool(name="f2n", bufs=L * CCH))
            raw_pool = ctx.enter_context(tc.tile_pool(name="raw", bufs=8))
            sq_pool = ctx.enter_context(tc.tile_pool(name="sq", bufs=4))
            row_pool = ctx.enter_context(tc.tile_pool(name="rows", bufs=6))
            rstg_note = None  # rstg tag tiles share row_pool
            invbc_pool = ctx.enter_context(tc.tile_pool(name="invbc", bufs=3))
            a1bc_pool = ctx.enter_context(tc.tile_pool(name="a1bc", bufs=13))
            a2bc_pool = ctx.enter_context(tc.tile_pool(name="a2bc", bufs=L))
            stt_pool = ctx.enter_context(tc.tile_pool(name="sttscr", bufs=2))
            cols_pool = ctx.enter_context(tc.tile_pool(name="cols", bufs=1))
            cst_pool = ctx.enter_context(tc.tile_pool(name="cst", bufs=2))
            fin_pool = ctx.enter_context(tc.tile_pool(name="fin", bufs=6))

            nc.gpsimd.load_library(library_config.lib)

            onescol = cst_pool.tile([128, 1], F32, tag="cst")
            nc.vector.memset(onescol[:], 1.0)
            mcol = cst_pool.tile([128, 1], F32, tag="cst")
            nc.vector.memset(mcol[:], 1.0 / HW)
            def bcast_row(src_dram_ap, name):
                stg = row_pool.tile([1, HW], F32, name=f"stg_{name}", tag="rstg")
                nc.sync.dma_start(stg[:], src_dram_ap)
                t = a1bc_pool.tile([128, HW], F32, name=f"bc_{name}", tag="a1bc")
                nc.gpsimd.partition_broadcast(t[:], stg[0:1, :], channels=128)
                return t

            cols2d = cols_pool.tile([128, L * 4 * 3 * S], F32)
            cols12 = cols_pool.tile([128, 3 * S * L], F32)
            if variant:
                nc.vector.memset(cols2d[:], 1.0)
                nc.vector.memset(cols12[:], 1.0)

            # a2 broadcast tiles (persist whole kernel)
            a2bc = []
            for l in range(L):
                stg = row_pool.tile([1, HW], F32, name=f"stga2_{l}", tag="rstg")
                nc.sync.dma_start(stg[:], a2r_d[l:l + 1, :])
                t = a2bc_pool.tile([128, HW], F32, name=f"a2bc{l}", tag="a2bc")
                nc.gpsimd.partition_broadcast(t[:], stg[0:1, :], channels=128)
                a2bc.append(t)

            f2nt = [[None] * CCH for _ in range(L)]

            def normalize_image(dst_tiles, src_ap_of_c):
                """DMA raw chunks, compute 1/max(||.||,eps) per pixel, write
                normalized chunks into dst_tiles."""
                raws = []
                nsq = pp.tile([1, HW], F32, tag="ps")
                for c in range(CCH):
                    rt = raw_pool.tile([128, HW], F32, name=f"raw{c}", tag="raw")
                    nc.sync.dma_start(rt[:], src_ap_of_c(c))
                    raws.append(rt)
                if "nonorm" in variant:
                    for c in range(CCH):
                        nc.scalar.activation(dst_tiles[c][:], raws[c][:], AF.Copy)
                    return
                for c in range(CCH):
                    sq = sq_pool.tile([128, HW], F32, name=f"sq{c}", tag="sq")
                    nc.scalar.activation(sq[:], raws[c][:], AF.Square)
                    nc.tensor.matmul(nsq[:], onescol[:, 0:1], sq[:],
                                     start=(c == 0), stop=(c == CCH - 1))
                nrm = row_pool.tile([1, HW], F32, tag="rows")
                nc.scalar.activation(nrm[:], nsq[0:1, :], AF.Sqrt)
                nrm2 = row_pool.tile([1, HW], F32, tag="rows")
                nc.vector.tensor_scalar_max(nrm2[:], nrm[:], 1e-12)
                inv = row_pool.tile([1, HW], F32, tag="rows")
                nc.vector.reciprocal(inv[:], nrm2[:])
                ibc = invbc_pool.tile([128, HW], F32, tag="invbc")
                nc.gpsimd.partition_broadcast(ibc[:], inv[0:1, :], channels=128)
                for c in range(CCH):
                    nc.gpsimd.tensor_tensor(dst_tiles[c][:], raws[c][:], ibc[:],
                                            op=OP.mult)

          # timing variants may repeat the whole pipeline to amortize RPC noise
          # (indentation kept flat on purpose via the loop below)
          # pylint: disable=redefined-outer-name
            for _rep in range(repeat):
             for bi, (ss0, bs) in enumerate(BLOCKS):
                # ---- normalization phase ----
                f1nt = {}
                for si in range(bs):
                    ss = ss0 + si
                    dst = [f1n_pool.tile([128, HW], F32, name=f"f1n_{ss}_{c}", tag="f1n") for c in range(CCH)]
                    normalize_image(
                        dst, lambda c, ss=ss: f1_d[ss, c * 128:(c + 1) * 128, :])
                    f1nt[ss] = dst
                if bi == 0:
                    for l in range(L):
                        dst = [f2n_pool.tile([128, HW], F32, name=f"f2n_{l}_{c}", tag="f2n") for c in range(CCH)]
                        normalize_image(
                            dst, lambda c, l=l: f2_d[l, c * 128:(c + 1) * 128, :])
                        f2nt[l] = dst

                # ---- T phase: T[y, x] per (l, ss); weights = f1n chunks ----
                for si in range(bs):
                    ss = ss0 + si
                    for pi, (y0, yp) in enumerate(PCH):
                        psT = pp.tile([yp, L, 512], F32, name="psT", tag="ps")
                        if "nomm" not in variant:
                            for c in range(CCH):
                                for l in range(L):
                                    nc.tensor.matmul(
                                        psT[:, l, 0:HW],
                                        f1nt[ss][c][:, y0:y0 + yp],
                                        f2nt[l][c][:, :],
                                        start=(c == 0), stop=(c == CCH - 1))
                        else:
                            nc.vector.memset(psT[:, :, :], 0.1)
                        if "nodve" in variant:
                            continue
                        o12 = (pi * S + ss) * L
                        nc.vector.reduce_max(cols12[0:yp, o12:o12 + L],
                                             psT[:, :, 0:HW], axis=AX_X)
                        for l in range(L):
                            og = _col_off(l, 3, pi, ss)
                            scr = stt_pool.tile([128, HW], F32, name="sttscr", tag="sttscr")
                            nc.vector.scalar_tensor_tensor(
                                scr[0:yp, :], psT[:, l, 0:HW],
                                cols12[0:yp, o12 + l:o12 + l + 1],
                                a2bc[l][0:yp, :],
                                op0=OP.is_ge, op1=OP.mult,
                                accum_out=cols2d[0:yp, og:og + 1])

                # ---- O phase: O[x, y] per (l, ss); weights = f2n chunks ----
                for l in range(L):
                    a1t = {}
                    for si in range(bs):
                        ss = ss0 + si
                        a1t[ss] = bcast_row(a1r_d[l, ss:ss + 1, :],
                                            f"{l}_{ss}")
                    for pi, (x0, xp) in enumerate(PCH):
                        for g0 in range(0, bs, GRP):
                            grp = [ss0 + si for si in range(g0, min(g0 + GRP, bs))]
                            ng = len(grp)
                            psO = pp.tile([xp, L, 512], F32, name="psO", tag="ps")
                            if "nomm" not in variant:
                                for c in range(CCH):
                                    for j, ss in enumerate(grp):
                                        nc.tensor.matmul(
                                            psO[:, j, 0:HW],
                                            f2nt[l][c][:, x0:x0 + xp],
                                            f1nt[ss][c][:, :],
                                            start=(c == 0), stop=(c == CCH - 1))
                            else:
                                nc.vector.memset(psO[:, :, :], 0.1)
                            if "nodve" in variant:
                                continue
                            # s21 for the ng consecutive ss: contiguous in cols2d
                            ob = _col_off(l, 0, pi, grp[0])
                            nc.vector.reduce_max(cols2d[0:xp, ob:ob + ng],
                                                 psO[:, 0:ng, 0:HW], axis=AX_X)
                            for j, ss in enumerate(grp):
                                og = _col_off(l, 2, pi, ss)
                                scr = stt_pool.tile([128, HW], F32, name="sttscr", tag="sttscr")
                                nc.vector.scalar_tensor_tensor(
                                    scr[0:xp, :], psO[:, j, 0:HW],
                                    cols2d[0:xp, ob + j:ob + j + 1],
                                    a1t[ss][0:xp, :],
                                    op0=OP.is_ge, op1=OP.mult,
                                    accum_out=cols2d[0:xp, og:og + 1])

             # ---- finals: w = g1*g2; out0 = mean(s12*w); out1 = mean(s21*w) ----
             for l in range(L):
                fp1 = pp.tile([1, S], F32, tag="ps")
                fp2 = pp.tile([1, S], F32, tag="ps")
                for pi, (p0, pn) in enumerate(PCH):
                    g1 = cols2d[0:pn, _col_off(l, 2, pi, 0):_col_off(l, 2, pi, 0) + S]
                    g2 = cols2d[0:pn, _col_off(l, 3, pi, 0):_col_off(l, 3, pi, 0) + S]
                    s21 = cols2d[0:pn, _col_off(l, 0, pi, 0):_col_off(l, 0, pi, 0) + S]
                    c12 = cols12[0:pn, :]
                    s12 = bass.AP(c12.tensor, c12.offset + pi * S * L + l,
                                  [c12.ap[0], [L, S]])
                    wt = fin_pool.tile([128, S], F32, tag="fin")
                    v1 = fin_pool.tile([128, S], F32, tag="fin")
                    v2 = fin_pool.tile([128, S], F32, tag="fin")
                    nc.vector.tensor_mul(wt[0:pn, :], g1, g2)
                    nc.vector.tensor_mul(v1[0:pn, :], s12, wt[0:pn, :])
                    nc.vector.tensor_mul(v2[0:pn, :], s21, wt[0:pn, :])
                    nc.tensor.matmul(fp1[:, :], mcol[0:pn, 0:1], v1[0:pn, :],
                                     start=(pi == 0), stop=(pi == 2))
                    nc.tensor.matmul(fp2[:, :], mcol[0:pn, 0:1], v2[0:pn, :],
                                     start=(pi == 0), stop=(pi == 2))
                st1 = fin_pool.tile([1, S], F32, name=f"st1_{l}", tag="finst")
                st2 = fin_pool.tile([1, S], F32, name=f"st2_{l}", tag="finst")
                nc.scalar.activation(st1[:], fp1[0:1, :], AF.Copy)
                nc.scalar.activation(st2[:], fp2[0:1, :], AF.Copy)
                nc.sync.dma_start(out_d[l:l + 1, :], st1[0:1, :])
                nc.sync.dma_start(out_d[L + l:L + l + 1, :], st2[0:1, :])

    nc.finalize()
    return nc


def _meta_learner_host(x, W1, g1, b1, m1, v1, W2, g2, b2, m2, v2):
    """x: [N, C, HW] -> [N, HW]  (two 1x1 convs + eval BN + ReLU on host)."""
    inv1 = g1 / np.sqrt(v1 + BN_EPS)
    bias1 = b1 - m1 * inv1
    y = np.einsum("tc,ncp->ntp", W1, x, dtype=np.float32)
    y = np.maximum(y * inv1[None, :, None] + bias1[None, :, None], 0.0)
    inv2 = g2 / np.sqrt(v2 + BN_EPS)
    bias2 = b2 - m2 * inv2
    z = np.einsum("ot,ntp->nop", W2, y, dtype=np.float32)
    z = np.maximum(z * inv2[None, :, None] + bias2[None, :, None], 0.0)
    return z[:, 0, :]


_NC_CACHE = [None]


def _prepare_in_maps(f1, f2, W1, g1, b1, m1, v1, W2, g2, b2, m2, v2):
    f1 = np.asarray(f1, np.float32).reshape(S, C, HW)
    f2 = np.asarray(f2, np.float32).reshape(Q, C, HW)
    W1 = np.asarray(W1, np.float32)
    W2 = np.asarray(W2, np.float32)
    g1, b1, m1, v1 = (np.asarray(a, np.float32) for a in (g1, b1, m1, v1))
    g2, b2, m2, v2 = (np.asarray(a, np.float32) for a in (g2, b2, m2, v2))

    # host meta-learner (tiny): a1 [S, HW], a2 [Q, HW]
    a1 = _meta_learner_host(f1, W1, g1, b1, m1, v1, W2, g2, b2, m2, v2)
    a2 = _meta_learner_host(f2, W1, g1, b1, m1, v1, W2, g2, b2, m2, v2)

    f2p = np.zeros((Q_PAD, C, HW), np.float32)
    f2p[:Q] = f2
    a2p = np.zeros((Q_PAD, HW), np.float32)
    a2p[:Q] = a2

    in_maps = []
    for core in range(NCORES):
        qq = [core * L + l for l in range(L)]
        a1r = np.zeros((L, S, HW), np.float32)
        a2r = np.zeros((L, HW), np.float32)
        for l, q in enumerate(qq):
            if q < Q:
                for ss in range(S):
                    i1 = (q * S + ss) // Q  # faithful torch-layout quirk
                    a1r[l, ss] = a1[i1]
                a2r[l] = a2p[q]
        in_maps.append({
            "f1": f1,
            "f2s": f2p[core * L:(core + 1) * L],
            "a1r": a1r,
            "a2r": a2r,
        })

    return in_maps


def _assemble(res):
    s1 = np.zeros((Q, S), np.float32)
    s2 = np.zeros((Q, S), np.float32)
    for core in range(NCORES):
        o = res.results[core]["out"].reshape(2, L, S)
        for l in range(L):
            q = core * L + l
            if q < Q:
                s1[q] = o[0, l]
                s2[q] = o[1, l]
    return np.stack([s1, s2])


def kernel(**inputs):
    in_maps = _prepare_in_maps(**inputs)
    if _NC_CACHE[0] is None:
        _NC_CACHE[0] = build_program()
    res = run_bass_kernel_spmd(_NC_CACHE[0], in_maps, list(range(NCORES)))
    return _assemble(res)



# revision 14
# speedup vs baseline: 1.6237x; 1.6237x over previous
"""Trainium2 Bass kernel for nn_CPSN (retrieval_knn PSM/PWG module).

Contract: kernel(**inputs) takes the FULL unsharded inputs (as produced by
setup_inputs) and returns the FULL output [2, b*q, s], distributing work
across 8 NeuronCores internally (data-parallel over the query dim q).

Algorithm per (q, s) pair (b=1, s=25, q=30, c=512, hw=361):
  O[x, y] = <f2n[:, x], f1n[:, y]>   (x = query pixel, y = support pixel)
  s21[x] = max_y O ; s12[y] = max_x O
  g1[x] = a1[argmax_y O[x, :]] ; g2[y] = a2[argmax_x O[:, y]]
  w = g1 * g2 ; out0 = mean(s12 * w) ; out1 = mean(s21 * w)

On device, argmax+gather is computed WITHOUT indices: the one-hot row
(O[x, :] == s21[x]) dotted with the (partition-broadcast) attention row a1
is exactly g1[x], via one fused DVE scalar_tensor_tensor pass.
Both orientations are computed by swapping matmul operands (PE contracts the
partition dim only).  The meta-learner (two 1x1 convs + BN + ReLU, ~1% of
FLOPs) runs on host; its output rows are fed pre-broadcast to the device.

Perf structure (v2):
  - All large matmuls run as float32r (TF32-class): 1 PE cycle/row at
    moving-dim >= 256 vs 4 for plain fp32.
  - Each PSUM tile is staged to SBUF by the (otherwise idle) Activation
    engine; the DVE stt ops then have all-SBUF operands, which enables the
    2x_2p DVE perf mode (half cycles), and PSUM buffers recycle faster.
  - DMAs are batched: one 3D-AP DMA per image, one staging DMA for all
    a1 attention rows.
"""

import os
import sys

import numpy as np

for _p in ("/opt/trn_rl_repo", "/root/.axon_site/_ro/trn_rl_repo"):
    if os.path.isdir(_p) and _p not in sys.path:
        sys.path.insert(0, _p)

import concourse.bass as bass
import concourse.tile as tile
from concourse import bacc, library_config, mybir
from concourse.bass_utils import run_bass_kernel_spmd

# ---- problem constants (hardcoded per contract) ----
B, S, Q, C, H, W, TEMP = 1, 25, 30, 512, 19, 19, 64
HW = H * W  # 361
HWP = HW + 1  # 362: fp32r matmuls need an even moving/dst free count
NCORES = 8
L = 4               # local (padded) query images per core; Q_PAD = 32
Q_PAD = NCORES * L
CCH = C // 128      # 4 contraction chunks
PCH = [(0, 128), (128, 128), (256, HW - 256)]  # pixel-dim partition chunks
BLOCKS = [(0, 9), (9, 8), (17, 8)]  # ss blocking to bound SBUF residency
GRP = 4             # O-phase ss group size (PSUM bank budget)
BN_EPS = 1e-5

F32 = mybir.dt.float32
F32R = mybir.dt.float32r
AX_X = mybir.AxisListType.X
OP = mybir.AluOpType
AF = mybir.ActivationFunctionType


def _col_off(l, kind, pch, ss):
    # cols2d free layout: [L][kind:4][pchunk:3][S]
    return ((l * 4 + kind) * 3 + pch) * S + ss


def build_program(variant="", repeat=1):
    """Build the (SPMD-shared) single-core bass program."""
    nc = bacc.Bacc(None, target_bir_lowering=False, debug=False)

    f1_d = nc.dram_tensor("f1", [S, C, HW], F32, kind="ExternalInput")
    f2_d = nc.dram_tensor("f2s", [L, C, HW], F32, kind="ExternalInput")
    # attention rows, host-gathered per (l, ss); broadcast to 128 rows on-device
    a1r_d = nc.dram_tensor("a1r", [L, S, HW], F32, kind="ExternalInput")
    a2r_d = nc.dram_tensor("a2r", [L, HW], F32, kind="ExternalInput")
    out_d = nc.dram_tensor("out", [2 * L, S], F32, kind="ExternalOutput")

    use_r = "f32" not in variant
    docopy = "nocopy" not in variant
    # PE runs in float32r (TF32-class) mode: 1 cycle/row vs 4 for fp32.
    # The verifier requires matmul operands to be *written* as float32r
    # (producer-side rounding), so operand tiles get MMT dtype.
    MMT = F32R if use_r else F32
    MMW = HWP if use_r else HW  # moving/dst free width for big matmuls

    def img_src_ap(dram_t, img):
        # [128, CCH, HW] view of one image: [p, cch, x] = t[img, cch*128+p, x]
        base = dram_t[img]
        return bass.AP(base.tensor, base.offset,
                       [[HW, 128], [128 * HW, CCH], [1, HW]])

    with tile.TileContext(nc) as tc:
        from contextlib import ExitStack

        with ExitStack() as ctx:
            pp = ctx.enter_context(tc.tile_pool(name="pp", bufs=2, space="PSUM"))
            raw_pool = ctx.enter_context(tc.tile_pool(name="raw", bufs=3))
            sq_pool = ctx.enter_context(tc.tile_pool(name="sq", bufs=2))
            f1n_pool = ctx.enter_context(
                tc.tile_pool(name="f1n", bufs=max(bs for _, bs in BLOCKS) * CCH))
            f2n_pool = ctx.enter_context(tc.tile_pool(name="f2n", bufs=L * CCH))
            ocp_pool = ctx.enter_context(tc.tile_pool(name="ocp", bufs=4))
            row_pool = ctx.enter_context(tc.tile_pool(name="rows", bufs=6))
            stg_pool = ctx.enter_context(tc.tile_pool(name="stg", bufs=8))
            invbc_pool = ctx.enter_context(tc.tile_pool(name="invbc", bufs=3))
            a1bc_pool = ctx.enter_context(
                tc.tile_pool(name="a1bc", bufs=max(bs for _, bs in BLOCKS)))
            a2bc_pool = ctx.enter_context(tc.tile_pool(name="a2bc", bufs=L))
            stt_pool = ctx.enter_context(tc.tile_pool(name="sttscr", bufs=2))
            cols_pool = ctx.enter_context(tc.tile_pool(name="cols", bufs=1))
            cst_pool = ctx.enter_context(tc.tile_pool(name="cst", bufs=2))
            fin_pool = ctx.enter_context(tc.tile_pool(name="fin", bufs=6))

            nc.gpsimd.load_library(library_config.lib)

            onesf = cst_pool.tile([128, 1], F32, tag="cst")
            nc.vector.memset(onesf[:], 1.0)
            onescol = cst_pool.tile([128, 1], MMT, tag="cst")
            nc.scalar.activation(onescol[:], onesf[:], AF.Copy)
            mcol = cst_pool.tile([128, 1], F32, tag="cst")
            nc.vector.memset(mcol[:], 1.0 / HW)

            cols2d = cols_pool.tile([128, L * 4 * 3 * S], F32)
            cols12 = cols_pool.tile([128, 3 * S * L], F32)
            if variant:
                nc.vector.memset(cols2d[:], 1.0)
                nc.vector.memset(cols12[:], 1.0)

            def bcast_row(src_dram_ap, name, pool, tag):
                # partition_broadcast needs its input at partition 0, so each
                # row gets its own [1, HW] staging tile
                stg = stg_pool.tile([1, HW], F32, name=f"stg_{name}", tag="stg")
                nc.sync.dma_start(stg[:], src_dram_ap)
                t = pool.tile([128, HW], F32, name=f"bc_{name}", tag=tag)
                nc.gpsimd.partition_broadcast(t[:], stg[0:1, :], channels=128)
                return t

            # a2 broadcast tiles (persist whole kernel)
            a2bc = [bcast_row(a2r_d[l:l + 1, :], f"a2_{l}", a2bc_pool, "a2bc")
                    for l in range(L)]

            f2nt = [[None] * CCH for _ in range(L)]

            def normalize_image(dst_tiles, dram_t, img):
                """DMA one image, compute 1/max(||.||,eps) per pixel, write
                normalized chunks into dst_tiles."""
                rawt = raw_pool.tile([128, CCH, HW], F32, name="rawt", tag="raw")
                nc.sync.dma_start(rawt[:], img_src_ap(dram_t, img))
                if "nonorm" in variant:
                    for c in range(CCH):
                        nc.scalar.activation(dst_tiles[c][:, 0:HW],
                                             rawt[:, c, :], AF.Copy)
                    return
                sq = sq_pool.tile([128, CCH, HWP], MMT, name="sq", tag="sq")
                nc.scalar.activation(sq[:, :, 0:HW], rawt[:], AF.Square)
                nsq = pp.tile([1, 512], F32, tag="ps")
                for c in range(CCH):
                    nc.tensor.matmul(nsq[:, 0:MMW], onescol[:, 0:1],
                                     sq[:, c, 0:MMW],
                                     start=(c == 0), stop=(c == CCH - 1))
                nrm = row_pool.tile([1, HW], F32, tag="rows")
                nc.scalar.activation(nrm[:], nsq[0:1, 0:HW], AF.Sqrt)
                nrm2 = row_pool.tile([1, HW], F32, tag="rows")
                nc.vector.tensor_scalar_max(nrm2[:], nrm[:], 1e-12)
                inv = row_pool.tile([1, HW], F32, tag="rows")
                nc.vector.reciprocal(inv[:], nrm2[:])
                ibc = invbc_pool.tile([128, HW], F32, tag="invbc")
                nc.gpsimd.partition_broadcast(ibc[:], inv[0:1, :], channels=128)
                for c in range(CCH):
                    nc.gpsimd.tensor_tensor(dst_tiles[c][:, 0:HW],
                                            rawt[:, c, :], ibc[:], op=OP.mult)

          # timing variants may repeat the whole pipeline to amortize RPC noise
          # (indentation kept flat on purpose via the loop below)
          # pylint: disable=redefined-outer-name
            for _rep in range(repeat):
             for bi, (ss0, bs) in enumerate(BLOCKS):
                # ---- normalization phase ----
                f1nt = {}
                for si in range(bs):
                    ss = ss0 + si
                    dst = [f1n_pool.tile([128, HWP], MMT, name=f"f1n_{ss}_{c}",
                                         tag="f1n") for c in range(CCH)]
                    normalize_image(dst, f1_d, ss)
                    f1nt[ss] = dst
                if bi == 0:
                    for l in range(L):
                        dst = [f2n_pool.tile([128, HWP], MMT,
                                             name=f"f2n_{l}_{c}", tag="f2n")
                               for c in range(CCH)]
                        normalize_image(dst, f2_d, l)
                        f2nt[l] = dst

                # ---- T phase: T[y, x] per (l, ss); weights = f1n chunks ----
                for si in range(bs):
                    ss = ss0 + si
                    for pi, (y0, yp) in enumerate(PCH):
                        psT = pp.tile([yp, L, 512], F32, name="psT", tag="ps")
                        if "nomm" not in variant:
                            for c in range(CCH):
                                for l in range(L):
                                    nc.tensor.matmul(
                                        psT[:, l, 0:MMW],
                                        f1nt[ss][c][:, y0:y0 + yp],
                                        f2nt[l][c][:, 0:MMW],
                                        start=(c == 0), stop=(c == CCH - 1))
                        else:
                            nc.vector.memset(psT[:, :, :], 0.1)
                        if "nodve" in variant:
                            continue
                        if docopy:
                            oc = ocp_pool.tile([128, L, HW], F32, name="ocT",
                                               tag="ocp")
                            nc.scalar.activation(oc[0:yp, :, :],
                                                 psT[:, :, 0:HW], AF.Copy)
                            rd = oc[0:yp, :, :]
                            rd_l = lambda l, oc=oc, yp=yp: oc[0:yp, l, :]
                        else:
                            rd = psT[:, :, 0:HW]
                            rd_l = lambda l, psT=psT: psT[:, l, 0:HW]
                        o12 = (pi * S + ss) * L
                        nc.vector.reduce_max(cols12[0:yp, o12:o12 + L],
                                             rd, axis=AX_X)
                        for l in range(L):
                            og = _col_off(l, 3, pi, ss)
                            scr = stt_pool.tile([128, HW], F32, name="sttscr",
                                                tag="sttscr")
                            nc.vector.scalar_tensor_tensor(
                                scr[0:yp, :], rd_l(l),
                                cols12[0:yp, o12 + l:o12 + l + 1],
                                a2bc[l][0:yp, :],
                                op0=OP.is_ge, op1=OP.mult,
                                accum_out=cols2d[0:yp, og:og + 1])

                # ---- O phase: O[x, y] per (l, ss); weights = f2n chunks ----
                for l in range(L):
                    a1t = {}
                    for si in range(bs):
                        ss = ss0 + si
                        a1t[ss] = bcast_row(a1r_d[l, ss:ss + 1, :],
                                            f"a1_{l}_{ss}", a1bc_pool, "a1bc")
                    for pi, (x0, xp) in enumerate(PCH):
                        for g0 in range(0, bs, GRP):
                            grp = [ss0 + si
                                   for si in range(g0, min(g0 + GRP, bs))]
                            ng = len(grp)
                            psO = pp.tile([xp, L, 512], F32, name="psO",
                                          tag="ps")
                            if "nomm" not in variant:
                                for c in range(CCH):
                                    for j, ss in enumerate(grp):
                                        nc.tensor.matmul(
                                            psO[:, j, 0:MMW],
                                            f2nt[l][c][:, x0:x0 + xp],
                                            f1nt[ss][c][:, 0:MMW],
                                            start=(c == 0), stop=(c == CCH - 1))
                            else:
                                nc.vector.memset(psO[:, :, :], 0.1)
                            if "nodve" in variant:
                                continue
                            if docopy:
                                oc = ocp_pool.tile([128, L, HW], F32,
                                                   name="ocO", tag="ocp")
                                nc.scalar.activation(oc[0:xp, 0:ng, :],
                                                     psO[:, 0:ng, 0:HW],
                                                     AF.Copy)
                                rd = oc[0:xp, 0:ng, :]
                                rd_j = lambda j, oc=oc, xp=xp: oc[0:xp, j, :]
                            else:
                                rd = psO[:, 0:ng, 0:HW]
                                rd_j = lambda j, psO=psO: psO[:, j, 0:HW]
                            # s21 for the ng consecutive ss: contiguous cols
                            ob = _col_off(l, 0, pi, grp[0])
                            nc.vector.reduce_max(cols2d[0:xp, ob:ob + ng],
                                                 rd, axis=AX_X)
                            for j, ss in enumerate(grp):
                                og = _col_off(l, 2, pi, ss)
                                scr = stt_pool.tile([128, HW], F32,
                                                    name="sttscr", tag="sttscr")
                                nc.vector.scalar_tensor_tensor(
                                    scr[0:xp, :], rd_j(j),
                                    cols2d[0:xp, ob + j:ob + j + 1],
                                    a1t[ss][0:xp, :],
                                    op0=OP.is_ge, op1=OP.mult,
                                    accum_out=cols2d[0:xp, og:og + 1])

             # ---- finals: w = g1*g2; out0 = mean(s12*w); out1 = mean(s21*w) ----
             for l in range(L):
                fp1 = pp.tile([1, S], F32, tag="ps")
                fp2 = pp.tile([1, S], F32, tag="ps")
                for pi, (p0, pn) in enumerate(PCH):
                    g1 = cols2d[0:pn, _col_off(l, 2, pi, 0):_col_off(l, 2, pi, 0) + S]
                    g2 = cols2d[0:pn, _col_off(l, 3, pi, 0):_col_off(l, 3, pi, 0) + S]
                    s21 = cols2d[0:pn, _col_off(l, 0, pi, 0):_col_off(l, 0, pi, 0) + S]
                    c12 = cols12[0:pn, :]
                    s12 = bass.AP(c12.tensor, c12.offset + pi * S * L + l,
                                  [c12.ap[0], [L, S]])
                    wt = fin_pool.tile([128, S], F32, tag="fin")
                    v1 = fin_pool.tile([128, S], F32, tag="fin")
                    v2 = fin_pool.tile([128, S], F32, tag="fin")
                    nc.vector.tensor_mul(wt[0:pn, :], g1, g2)
                    nc.vector.tensor_mul(v1[0:pn, :], s12, wt[0:pn, :])
                    nc.vector.tensor_mul(v2[0:pn, :], s21, wt[0:pn, :])
                    nc.tensor.matmul(fp1[:, :], mcol[0:pn, 0:1], v1[0:pn, :],
                                     start=(pi == 0), stop=(pi == 2))
                    nc.tensor.matmul(fp2[:, :], mcol[0:pn, 0:1], v2[0:pn, :],
                                     start=(pi == 0), stop=(pi == 2))
                st1 = fin_pool.tile([1, S], F32, name=f"st1_{l}", tag="finst")
                st2 = fin_pool.tile([1, S], F32, name=f"st2_{l}", tag="finst")
                nc.scalar.activation(st1[:], fp1[0:1, :], AF.Copy)
                nc.scalar.activation(st2[:], fp2[0:1, :], AF.Copy)
                nc.sync.dma_start(out_d[l:l + 1, :], st1[0:1, :])
                nc.sync.dma_start(out_d[L + l:L + l + 1, :], st2[0:1, :])

    nc.finalize()
    return nc


def _meta_learner_host(x, W1, g1, b1, m1, v1, W2, g2, b2, m2, v2):
    """x: [N, C, HW] -> [N, HW]  (two 1x1 convs + eval BN + ReLU on host)."""
    inv1 = g1 / np.sqrt(v1 + BN_EPS)
    bias1 = b1 - m1 * inv1
    y = np.einsum("tc,ncp->ntp", W1, x, dtype=np.float32)
    y = np.maximum(y * inv1[None, :, None] + bias1[None, :, None], 0.0)
    inv2 = g2 / np.sqrt(v2 + BN_EPS)
    bias2 = b2 - m2 * inv2
    z = np.einsum("ot,ntp->nop", W2, y, dtype=np.float32)
    z = np.maximum(z * inv2[None, :, None] + bias2[None, :, None], 0.0)
    return z[:, 0, :]


_NC_CACHE = [None]


def _prepare_in_maps(f1, f2, W1, g1, b1, m1, v1, W2, g2, b2, m2, v2):
    f1 = np.asarray(f1, np.float32).reshape(S, C, HW)
    f2 = np.asarray(f2, np.float32).reshape(Q, C, HW)
    W1 = np.asarray(W1, np.float32)
    W2 = np.asarray(W2, np.float32)
    g1, b1, m1, v1 = (np.asarray(a, np.float32) for a in (g1, b1, m1, v1))
    g2, b2, m2, v2 = (np.asarray(a, np.float32) for a in (g2, b2, m2, v2))

    # host meta-learner (tiny): a1 [S, HW], a2 [Q, HW]
    a1 = _meta_learner_host(f1, W1, g1, b1, m1, v1, W2, g2, b2, m2, v2)
    a2 = _meta_learner_host(f2, W1, g1, b1, m1, v1, W2, g2, b2, m2, v2)

    f2p = np.zeros((Q_PAD, C, HW), np.float32)
    f2p[:Q] = f2
    a2p = np.zeros((Q_PAD, HW), np.float32)
    a2p[:Q] = a2

    in_maps = []
    for core in range(NCORES):
        qq = [core * L + l for l in range(L)]
        a1r = np.zeros((L, S, HW), np.float32)
        a2r = np.zeros((L, HW), np.float32)
        for l, q in enumerate(qq):
            if q < Q:
                for ss in range(S):
                    i1 = (q * S + ss) // Q  # faithful torch-layout quirk
                    a1r[l, ss] = a1[i1]
                a2r[l] = a2p[q]
        in_maps.append({
            "f1": f1,
            "f2s": f2p[core * L:(core + 1) * L],
            "a1r": a1r,
            "a2r": a2r,
        })

    return in_maps


def _assemble(res):
    s1 = np.zeros((Q, S), np.float32)
    s2 = np.zeros((Q, S), np.float32)
    for core in range(NCORES):
        o = res.results[core]["out"].reshape(2, L, S)
        for l in range(L):
            q = core * L + l
            if q < Q:
                s1[q] = o[0, l]
                s2[q] = o[1, l]
    return np.stack([s1, s2])


def kernel(**inputs):
    in_maps = _prepare_in_maps(**inputs)
    if _NC_CACHE[0] is None:
        _NC_CACHE[0] = build_program()
    res = run_bass_kernel_spmd(_NC_CACHE[0], in_maps, list(range(NCORES)))
    return _assemble(res)
